# revision 11
# baseline (speedup 1.0000x reference)
"""Bass/Tile kernel: cosine top-20 adjacency (16384x64 embeddings) on 8 trn2 cores.

Per-core algorithm (rows sharded 2048/core via host-side input rotation, so the
same SPMD graph runs on every core):
  1. Load embeddings row-major, compute row norms (square -> windowed reduce ->
     sqrt -> reciprocal), fused normalize+bf16-cast.
  2. Round-trip through DRAM and XBAR-transpose the [8192, 128] bf16 view ->
     normT [64, 16384] with columns permuted to [even rows | odd rows]
     (column order is irrelevant: the output is values-only).
  3. Per 128-row tile (t<8: even local rows of band t; t>=8: odd rows):
     sim = lhsT.T @ normT (bf16 matmuls into PSUM). PSUM evacuation per
     2048-col group: A-groups - Act casts all 2048 to bf16, DVE tensor_max
     folds halves (2x mode); H-groups - Act casts the hi 1024, DVE folds
     psum-lo against it (HW allows one PSUM operand). Fold pyramid to 512
     windowed maxima, max8 per 128-chunk -> 32 candidates,
     3x(max8 + match_replace) -> top-24 descending.
  4. Self-similarity (~1.0) is always the strict row max, so
     out[:,0] = 0 and out[:,1:20] = sigmoid(top24[:,1:20]).
"""

import os
import sys

import numpy as np

for _p in ("/opt/trn_rl_repo",):
    if _p not in sys.path and os.path.isdir(_p):
        sys.path.insert(0, _p)

import concourse.bass as bass  # noqa: E402
import concourse.mybir as mybir  # noqa: E402
import concourse.tile as tile  # noqa: E402
from concourse import bacc  # noqa: E402
from concourse.bass_utils import run_bass_kernel_spmd  # noqa: E402

N = 16384
D = 64
TOPK = 20
CORES = 8
R = N // CORES  # 2048 rows per core
T = R // 128  # 16 row tiles per core
G = 2048  # column group size
NG = N // G  # 8 column groups
H = N // 2  # even/odd half size in permuted column space
NEG = -1.0e30

f32 = mybir.dt.float32
bf16 = mybir.dt.bfloat16
AF = mybir.ActivationFunctionType
ALU = mybir.AluOpType

# Per-group evacuation: True = H-path (Act casts hi half, DVE folds psum-lo
# against it at 1x), False = A-path (Act casts all 2048, DVE folds at 2x).
H_GROUP = (False, False, True, False, True, False, True, False)

_CACHE = {}


def _build_nc():
    nc = bacc.Bacc(
        "TRN2", target_bir_lowering=False, debug=False, enable_asserts=False
    )
    emb = nc.dram_tensor("embeddings", [N, D], f32, kind="ExternalInput")
    out = nc.dram_tensor("out", [R, TOPK], f32, kind="ExternalOutput")
    # lhsT for tile t is a contiguous permuted-column slice; tile t<8 covers
    # even local rows 2*(t*128+q), tile t>=8 covers odd rows 2*((t-8)*128+q)+1.
    # This view un-permutes on the output DMA: out_v[h, j] = local row 2j+h.
    out_v = out[:].rearrange("(j two) k -> two j k", two=2)

    with tile.TileContext(nc) as tc:
        with tc.tile_pool(name="persist", bufs=1) as persist:
            normT = persist.tile([D, N], bf16)

            # ---- Prologue: normalize rows, cast bf16, XBAR transpose ----
            # Split into halves so the stages pipeline.
            with (
                tc.tile_pool(name="pro_rm", bufs=1) as pro_rm,
                tc.tile_pool(name="pro_t2", bufs=1) as pro_t2,
                tc.tile_pool(name="pro_dram", bufs=1, space="DRAM") as pro_dram,
            ):
                # flat [128, 128, 64] staging view: row r = p*128 + a
                emb_v = emb[:].rearrange("(p a) d -> p a d", p=128)
                rm = pro_rm.tile([128, 128, D], f32)
                sq = pro_rm.tile([128, 128, D], f32)
                ssq = pro_rm.tile([128, 128], f32)
                slen = pro_rm.tile([128, 128], f32)
                sinv = pro_rm.tile([128, 128], f32)
                rmb = pro_rm.tile([128, 128, D], bf16)
                scratch = pro_dram.tile([N, D], bf16)
                sc_v = scratch[:].rearrange("(p a) d -> p a d", p=128)
                engs = (nc.sync, nc.scalar)
                for c in range(2):
                    cs = slice(c * 64, (c + 1) * 64)
                    engs[c].dma_start(rm[:, cs, :], emb_v[:, cs, :])
                    nc.scalar.activation(sq[:, cs, :], rm[:, cs, :], AF.Square)
                    nc.vector.tensor_reduce(
                        ssq[:, cs], sq[:, cs, :],
                        axis=mybir.AxisListType.X, op=ALU.add,
                    )
                    nc.scalar.activation(slen[:, cs], ssq[:, cs], AF.Sqrt)
                    nc.vector.reciprocal(sinv[:, cs], slen[:, cs])
                    nc.vector.scalar_tensor_tensor(
                        rmb[:, cs, :], rm[:, cs, :], 1.0,
                        sinv[:, cs].to_broadcast((128, 64, D)),
                        op0=ALU.mult, op1=ALU.mult,
                    )
                    engs[c].dma_start(sc_v[:, cs, :], rmb[:, cs, :])

                # XBAR transpose of the [8192, 128] bf16 view, in two row
                # chunks: nt2 partition c<64 holds column c over even rows,
                # 64+d over odd rows.
                sc_t = scratch[:].rearrange("(m two) d -> m (two d)", two=2)
                nt2 = pro_t2.tile([128, H], bf16)
                for c in range(2):
                    ms = slice(c * 4096, (c + 1) * 4096)
                    nc.scalar.dma_start(
                        out=nt2[:, ms], in_=sc_t[ms, :], transpose=True
                    )
                    nc.vector.tensor_copy(
                        normT[:, c * 4096 : (c + 1) * 4096], nt2[0:D, ms]
                    )
                    nc.sync.dma_start(
                        normT[:, H + c * 4096 : H + (c + 1) * 4096],
                        nt2[D:128, ms],
                    )

            # ---- Main loop: 16 row tiles ----
            with (
                tc.tile_pool(name="mm_psum", bufs=2, space="PSUM") as mm_psum,
                tc.tile_pool(name="ev_cast", bufs=6) as ev_cast,
                tc.tile_pool(name="ev_ch", bufs=5) as ev_ch,
                tc.tile_pool(name="pyr", bufs=2) as pyr,
                tc.tile_pool(name="fin", bufs=2) as fin,
            ):
                for t in range(T):
                    # tile t<8: even local rows; t>=8: odd local rows
                    c0 = t * 128 if t < 8 else H + (t - 8) * 128
                    lhsT = normT[:, c0 : c0 + 128]
                    l1b = pyr.tile([128, NG, G // 2], bf16, tag="l1b")
                    for g in range(NG):
                        ps = mm_psum.tile([128, G], f32, tag="ps")
                        for s in range(G // 512):
                            cs = slice(g * G + s * 512, g * G + (s + 1) * 512)
                            nc.tensor.matmul(
                                ps[:, s * 512 : (s + 1) * 512],
                                lhsT,
                                normT[:, cs],
                            )
                        if H_GROUP[g]:
                            ch = ev_ch.tile([128, G // 2], bf16, tag="ch")
                            nc.scalar.activation(
                                ch[:], ps[:, G // 2 : G], AF.Copy
                            )
                            nc.vector.tensor_max(
                                l1b[:, g, :], ps[:, 0 : G // 2], ch[:]
                            )
                        else:
                            ca = ev_cast.tile([128, G], bf16, tag="ca")
                            nc.scalar.activation(ca[:], ps[:], AF.Copy)
                            nc.vector.tensor_max(
                                l1b[:, g, :],
                                ca[:, 0 : G // 2], ca[:, G // 2 : G],
                            )

                    # fold pyramid: 8x1024 -> 4096 -> 2048 -> 1024 -> 512
                    f2 = pyr.tile([128, 4, G // 2], bf16, tag="f2")
                    nc.vector.tensor_max(f2[:], l1b[:, 0:4, :], l1b[:, 4:8, :])
                    f3 = pyr.tile([128, 2, G // 2], bf16, tag="f3")
                    nc.vector.tensor_max(f3[:], f2[:, 0:2, :], f2[:, 2:4, :])
                    f4 = pyr.tile([128, G // 2], bf16, tag="f4")
                    nc.vector.tensor_max(f4[:], f3[:, 0, :], f3[:, 1, :])
                    f5 = pyr.tile([128, G // 4], bf16, tag="f5")
                    nc.vector.tensor_max(
                        f5[:], f4[:, 0 : G // 4], f4[:, G // 4 : G // 2]
                    )

                    # candidates: top-8 of each 128-chunk of the 512 maxima
                    cand = fin.tile([128, 32], bf16, tag="cand")
                    for c in range(4):
                        nc.vector.max(
                            out=cand[:, c * 8 : (c + 1) * 8],
                            in_=f5[:, c * 128 : (c + 1) * 128],
                        )
                    # top-24 via 3x max8 + 2x match_replace
                    top24 = fin.tile([128, 24], bf16, tag="top24")
                    cand2 = fin.tile([128, 32], bf16, tag="cand2")
                    cand3 = fin.tile([128, 32], bf16, tag="cand3")
                    nc.vector.max(out=top24[:, 0:8], in_=cand[:])
                    nc.vector.match_replace(
                        out=cand2[:], in_to_replace=top24[:, 0:8],
                        in_values=cand[:], imm_value=NEG,
                    )
                    nc.vector.max(out=top24[:, 8:16], in_=cand2[:])
                    nc.vector.match_replace(
                        out=cand3[:], in_to_replace=top24[:, 8:16],
                        in_values=cand2[:], imm_value=NEG,
                    )
                    nc.vector.max(out=top24[:, 16:24], in_=cand3[:])

                    # epilogue: out[:,0] = 0; out[:,1:20] = sigmoid(top24[:,1:20])
                    osb = fin.tile([128, TOPK], f32, tag="osb")
                    nc.gpsimd.memset(osb[:, 0:1], 0.0)
                    nc.scalar.activation(
                        osb[:, 1:TOPK], top24[:, 1:TOPK], AF.Sigmoid
                    )
                    hh, band = (0, t) if t < 8 else (1, t - 8)
                    nc.sync.dma_start(
                        out_v[hh, band * 128 : (band + 1) * 128, :], osb[:]
                    )

    nc.compile()
    return nc


def get_nc():
    if "nc" not in _CACHE:
        _CACHE["nc"] = _build_nc()
    return _CACHE["nc"]


def kernel(embeddings: np.ndarray) -> np.ndarray:
    emb = np.ascontiguousarray(np.asarray(embeddings, dtype=np.float32))
    assert emb.shape == (N, D), emb.shape
    nc = get_nc()
    in_maps = [
        {"embeddings": np.roll(emb, -i * R, axis=0)} for i in range(CORES)
    ]
    res = run_bass_kernel_spmd(nc, in_maps, core_ids=list(range(CORES)))
    _CACHE["last_results"] = res
    return np.concatenate(
        [res.results[i]["out"] for i in range(CORES)], axis=0
    ).astype(np.float32)


# revision 12
# speedup vs baseline: 1.2885x; 1.2885x over previous
"""Bass/Tile kernel: cosine top-20 adjacency (16384x64 embeddings) on 8 trn2 cores.

Per-core algorithm (rows sharded 2048/core via host-side input rotation, so the
same SPMD graph runs on every core):
  1. Load embeddings row-major, compute row norms (square -> windowed reduce ->
     sqrt -> reciprocal), fused normalize+bf16-cast.
  2. Round-trip through DRAM and XBAR-transpose the [8192, 128] bf16 view ->
     normT [64, 16384] with columns permuted to [even rows | odd rows]
     (column order is irrelevant: the output is values-only).
  3. Per 128-row tile (t<8: even local rows of band t; t>=8: odd rows):
     sim = lhsT.T @ normT (bf16 matmuls into PSUM). PSUM evacuation per
     2048-col group: A-groups - Act casts all 2048 to bf16, DVE tensor_max
     folds halves (2x mode); H-groups - Act casts the hi 1024, DVE folds
     psum-lo against it (HW allows one PSUM operand). Fold pyramid to 512
     windowed maxima, max8 per 128-chunk -> 32 candidates,
     3x(max8 + match_replace) -> top-24 descending.
  4. Self-similarity (~1.0) is always the strict row max, so
     out[:,0] = 0 and out[:,1:20] = sigmoid(top24[:,1:20]).
"""

import os
import sys

import numpy as np

for _p in ("/opt/trn_rl_repo",):
    if _p not in sys.path and os.path.isdir(_p):
        sys.path.insert(0, _p)

import concourse.bass as bass  # noqa: E402
import concourse.mybir as mybir  # noqa: E402
import concourse.tile as tile  # noqa: E402
from concourse import bacc  # noqa: E402
from concourse.bass_utils import run_bass_kernel_spmd  # noqa: E402

N = 16384
D = 64
TOPK = 20
CORES = 8
R = N // CORES  # 2048 rows per core
T = R // 128  # 16 row tiles per core
G = 2048  # column group size
NG = N // G  # 8 column groups
H = N // 2  # even/odd half size in permuted column space
NEG = -1.0e30

f32 = mybir.dt.float32
bf16 = mybir.dt.bfloat16
AF = mybir.ActivationFunctionType
ALU = mybir.AluOpType

# Per-group evacuation: True = H-path (Act casts hi half, DVE folds psum-lo
# against it at 1x), False = A-path (Act casts all 2048, DVE folds at 2x).
H_GROUP = (False, False, False, False, False, False, False, False)

_CACHE = {}


def _build_nc():
    nc = bacc.Bacc(
        "TRN2", target_bir_lowering=False, debug=False, enable_asserts=False
    )
    emb = nc.dram_tensor("embeddings", [N, D], f32, kind="ExternalInput")
    out = nc.dram_tensor("out", [R, TOPK], f32, kind="ExternalOutput")
    # lhsT for tile t is a contiguous permuted-column slice; tile t<8 covers
    # even local rows 2*(t*128+q), tile t>=8 covers odd rows 2*((t-8)*128+q)+1.
    # This view un-permutes on the output DMA: out_v[h, j] = local row 2j+h.
    out_v = out[:].rearrange("(j two) k -> two j k", two=2)

    with tile.TileContext(nc) as tc:
        with tc.tile_pool(name="persist", bufs=1) as persist:
            normT = persist.tile([D, N], bf16)

            # ---- Prologue: normalize rows, cast bf16, XBAR transpose ----
            # Split into halves so the stages pipeline.
            with (
                tc.tile_pool(name="pro_rm", bufs=1) as pro_rm,
                tc.tile_pool(name="pro_t2", bufs=1) as pro_t2,
                tc.tile_pool(name="pro_dram", bufs=1, space="DRAM") as pro_dram,
            ):
                # flat [128, 128, 64] staging view: row r = p*128 + a
                emb_v = emb[:].rearrange("(p a) d -> p a d", p=128)
                rm = pro_rm.tile([128, 128, D], f32)
                sq = pro_rm.tile([128, 128, D], f32)
                ssq = pro_rm.tile([128, 128], f32)
                slen = pro_rm.tile([128, 128], f32)
                sinv = pro_rm.tile([128, 128], f32)
                rmb = pro_rm.tile([128, 128, D], bf16)
                scratch = pro_dram.tile([N, D], bf16)
                sc_v = scratch[:].rearrange("(p a) d -> p a d", p=128)
                engs = (nc.sync, nc.scalar)
                for c in range(2):
                    cs = slice(c * 64, (c + 1) * 64)
                    engs[c].dma_start(rm[:, cs, :], emb_v[:, cs, :])
                    nc.scalar.activation(sq[:, cs, :], rm[:, cs, :], AF.Square)
                    nc.vector.tensor_reduce(
                        ssq[:, cs], sq[:, cs, :],
                        axis=mybir.AxisListType.X, op=ALU.add,
                    )
                    nc.scalar.activation(slen[:, cs], ssq[:, cs], AF.Sqrt)
                    nc.vector.reciprocal(sinv[:, cs], slen[:, cs])
                    nc.vector.scalar_tensor_tensor(
                        rmb[:, cs, :], rm[:, cs, :], 1.0,
                        sinv[:, cs].to_broadcast((128, 64, D)),
                        op0=ALU.mult, op1=ALU.mult,
                    )
                    engs[c].dma_start(sc_v[:, cs, :], rmb[:, cs, :])

                # XBAR transpose of the [8192, 128] bf16 view, in two row
                # chunks: nt2 partition c<64 holds column c over even rows,
                # 64+d over odd rows.
                sc_t = scratch[:].rearrange("(m two) d -> m (two d)", two=2)
                nt2 = pro_t2.tile([128, H], bf16)
                for c in range(2):
                    ms = slice(c * 4096, (c + 1) * 4096)
                    nc.scalar.dma_start(
                        out=nt2[:, ms], in_=sc_t[ms, :], transpose=True
                    )
                    nc.vector.tensor_copy(
                        normT[:, c * 4096 : (c + 1) * 4096], nt2[0:D, ms]
                    )
                    nc.sync.dma_start(
                        normT[:, H + c * 4096 : H + (c + 1) * 4096],
                        nt2[D:128, ms],
                    )

            # ---- Main loop: 16 row tiles ----
            with (
                tc.tile_pool(name="mm_psum", bufs=2, space="PSUM") as mm_psum,
                tc.tile_pool(name="ev_cast", bufs=6) as ev_cast,
                tc.tile_pool(name="ev_ch", bufs=5) as ev_ch,
                tc.tile_pool(name="pyr", bufs=2) as pyr,
                tc.tile_pool(name="fin", bufs=2) as fin,
            ):
                for t in range(T):
                    # tile t<8: even local rows; t>=8: odd local rows
                    c0 = t * 128 if t < 8 else H + (t - 8) * 128
                    lhsT = normT[:, c0 : c0 + 128]
                    l1b = pyr.tile([128, NG, G // 2], bf16, tag="l1b")
                    for g in range(NG):
                        ps = mm_psum.tile([128, G], f32, tag="ps")
                        for s in range(G // 512):
                            cs = slice(g * G + s * 512, g * G + (s + 1) * 512)
                            nc.tensor.matmul(
                                ps[:, s * 512 : (s + 1) * 512],
                                lhsT,
                                normT[:, cs],
                            )
                        if H_GROUP[g]:
                            ch = ev_ch.tile([128, G // 2], bf16, tag="ch")
                            nc.scalar.activation(
                                ch[:], ps[:, G // 2 : G], AF.Copy
                            )
                            nc.vector.tensor_max(
                                l1b[:, g, :], ps[:, 0 : G // 2], ch[:]
                            )
                        else:
                            ca = ev_cast.tile([128, G], bf16, tag="ca")
                            nc.scalar.activation(ca[:], ps[:], AF.Copy)
                            nc.vector.tensor_max(
                                l1b[:, g, :],
                                ca[:, 0 : G // 2], ca[:, G // 2 : G],
                            )

                    # fold pyramid: 8x1024 -> 4096 -> 2048 -> 1024 -> 512
                    f2 = pyr.tile([128, 4, G // 2], bf16, tag="f2")
                    nc.vector.tensor_max(f2[:], l1b[:, 0:4, :], l1b[:, 4:8, :])
                    f3 = pyr.tile([128, 2, G // 2], bf16, tag="f3")
                    nc.vector.tensor_max(f3[:], f2[:, 0:2, :], f2[:, 2:4, :])
                    f4 = pyr.tile([128, G // 2], bf16, tag="f4")
                    nc.vector.tensor_max(f4[:], f3[:, 0, :], f3[:, 1, :])
                    f5 = pyr.tile([128, G // 4], bf16, tag="f5")
                    nc.vector.tensor_max(
                        f5[:], f4[:, 0 : G // 4], f4[:, G // 4 : G // 2]
                    )

                    # candidates: top-8 of each 128-chunk of the 512 maxima
                    cand = fin.tile([128, 32], bf16, tag="cand")
                    for c in range(4):
                        nc.vector.max(
                            out=cand[:, c * 8 : (c + 1) * 8],
                            in_=f5[:, c * 128 : (c + 1) * 128],
                        )
                    # top-24 via 3x max8 + 2x match_replace
                    top24 = fin.tile([128, 24], bf16, tag="top24")
                    cand2 = fin.tile([128, 32], bf16, tag="cand2")
                    cand3 = fin.tile([128, 32], bf16, tag="cand3")
                    nc.vector.max(out=top24[:, 0:8], in_=cand[:])
                    nc.vector.match_replace(
                        out=cand2[:], in_to_replace=top24[:, 0:8],
                        in_values=cand[:], imm_value=NEG,
                    )
                    nc.vector.max(out=top24[:, 8:16], in_=cand2[:])
                    nc.vector.match_replace(
                        out=cand3[:], in_to_replace=top24[:, 8:16],
                        in_values=cand2[:], imm_value=NEG,
                    )
                    nc.vector.max(out=top24[:, 16:24], in_=cand3[:])

                    # epilogue: out[:,0] = 0; out[:,1:20] = sigmoid(top24[:,1:20])
                    osb = fin.tile([128, TOPK], f32, tag="osb")
                    nc.gpsimd.memset(osb[:, 0:1], 0.0)
                    nc.scalar.activation(
                        osb[:, 1:TOPK], top24[:, 1:TOPK], AF.Sigmoid
                    )
                    hh, band = (0, t) if t < 8 else (1, t - 8)
                    nc.sync.dma_start(
                        out_v[hh, band * 128 : (band + 1) * 128, :], osb[:]
                    )

    nc.compile()
    return nc


def get_nc():
    if "nc" not in _CACHE:
        _CACHE["nc"] = _build_nc()
    return _CACHE["nc"]


def kernel(embeddings: np.ndarray) -> np.ndarray:
    emb = np.ascontiguousarray(np.asarray(embeddings, dtype=np.float32))
    assert emb.shape == (N, D), emb.shape
    nc = get_nc()
    in_maps = [
        {"embeddings": np.roll(emb, -i * R, axis=0)} for i in range(CORES)
    ]
    res = run_bass_kernel_spmd(nc, in_maps, core_ids=list(range(CORES)))
    _CACHE["last_results"] = res
    return np.concatenate(
        [res.results[i]["out"] for i in range(CORES)], axis=0
    ).astype(np.float32)
